# revision 18
# baseline (speedup 1.0000x reference)
# Multi-head attention (B=2, S=2048, H=1024, NH=16) on 8 TRN2 NeuronCores.
#
# Sharding: core c handles batch b = c // 4 and head group hg = c % 4
# (4 heads of 64 dims each => a 256-wide slice of the projection output).
# Each core computes, for its (b, hg):
#   Q^T/K^T head tiles (augmented with a ones/mask-bias contraction row so
#   the additive key mask comes out of the PE for free), V in natural
#   [s, d] layout, scores = Q.K/8 + maskbias, softmax along the free dim
#   (no max subtraction -- scores are O(10) so exp cannot overflow, and
#   masked lanes carry -1e30 which underflows exp to exactly 0), the
#   normalized attention tile (DMAed out: it is a required output), P@V via
#   PE transposes of P, and a partial output projection against the
#   matching 256-row slice of Wo.  The host sums the 4 partial output
#   projections per batch (row-parallel Wo => host all-reduce) and adds
#   the Wo bias.
import sys
from contextlib import ExitStack

import numpy as np

if "/opt/trn_rl_repo" not in sys.path:
    sys.path.insert(0, "/opt/trn_rl_repo")

import concourse.bacc as bacc
import concourse.bass as bass
import concourse.mybir as mybir
import concourse.tile as tile
from concourse import masks
from concourse.bass_utils import run_bass_kernel_spmd

F32 = mybir.dt.float32
F32R = mybir.dt.float32r
AF = mybir.ActivationFunctionType
AX = mybir.AxisListType

B, S, H, NH = 2, 2048, 1024, 16
DK = H // NH
HL = NH // 4          # heads per core
DL = HL * DK          # local projection width
NEG = -1.0e30


def _r(ap):
    return ap.bitcast(F32R)


def build_nc(S=S, H=H, HL=HL, DK=DK, no_attn_dma=False, no_pv=False, no_sm=False, no_proj=False, no_b=False, no_a=False):
    KC = H // 128         # contraction chunks over the input dim
    ST = S // 128         # 128-row tiles over the sequence
    DL = HL * DK
    NW = max(1, S // 512)  # 512-wide N chunks for the Q/K projections
    PW = min(S, 512)      # scores psum chunk width (1 bank)
    NPS = S // PW
    NG = 4                # q-tiles per PV group
    KT2 = S // 128

    nc = bacc.Bacc("TRN2", target_bir_lowering=False)

    qT = nc.dram_tensor("qT", [H, S], F32, kind="ExternalInput")
    kT = nc.dram_tensor("kT", [H, S], F32, kind="ExternalInput")
    vT = nc.dram_tensor("vT", [H, S], F32, kind="ExternalInput")
    wqT = nc.dram_tensor("wqT", [H, DL], F32, kind="ExternalInput")
    wkT = nc.dram_tensor("wkT", [H, DL], F32, kind="ExternalInput")
    wvT = nc.dram_tensor("wvT", [H, DL], F32, kind="ExternalInput")
    woT = nc.dram_tensor("woT", [DL, H], F32, kind="ExternalInput")
    bqs = nc.dram_tensor("bqs", [DK, HL], F32, kind="ExternalInput")
    bks = nc.dram_tensor("bks", [DK, HL], F32, kind="ExternalInput")
    bvb = nc.dram_tensor("bvb", [128, DL], F32, kind="ExternalInput")
    mbias = nc.dram_tensor("mbias", [1, S], F32, kind="ExternalInput")
    onesd = nc.dram_tensor("onesd", [1, S], F32, kind="ExternalInput")
    eyed = nc.dram_tensor("eyed", [128, 128], F32, kind="ExternalInput")

    attn_o = nc.dram_tensor("attn_o", [HL, S, S], F32, kind="ExternalOutput")
    out_o = nc.dram_tensor("out_o", [S, H], F32, kind="ExternalOutput")

    with tile.TileContext(nc) as tc, ExitStack() as top:
        const = top.enter_context(tc.tile_pool(name="const", bufs=1))
        ident = const.tile([128, 128], F32, name="ident", tag="ident")
        nc.sync.dma_start(ident[:], eyed[:])
        identr = const.tile([128, 128], F32R, name="identr", tag="identr")
        nc.sync.dma_start(identr[:], eyed[:].bitcast(F32R))
        ones_sb = const.tile([1, 64], F32R, name="ones_sb", tag="ones_sb")
        nc.sync.dma_start(ones_sb[:], onesd[0:1, 0:64].bitcast(F32R))
        bq_sb = const.tile([DK, HL], F32, name="bq", tag="bq")
        bk_sb = const.tile([DK, HL], F32, name="bk", tag="bk")
        bvb_sb = const.tile([128, DL], F32, name="bvb", tag="bvb")
        nc.sync.dma_start(bq_sb[:], bqs[:])
        nc.sync.dma_start(bk_sb[:], bks[:])
        nc.sync.dma_start(bvb_sb[:], bvb[:])
        wo_sb = [const.tile([64, H], F32R, name=f"wo{i}", tag=f"wo{i}") for i in range(HL)]
        for i in range(HL):
            nc.sync.dma_start(wo_sb[i][:], woT[i * DK:(i + 1) * DK, :].bitcast(F32R))

        pers = top.enter_context(tc.tile_pool(name="pers", bufs=1))
        QTa = [pers.tile([65, S], F32R, name=f"QTa{h}", tag=f"QTa{h}") for h in range(HL)]
        KTa = [pers.tile([65, S], F32R, name=f"KTa{h}", tag=f"KTa{h}") for h in range(HL)]
        V_sb = [pers.tile([128, DL], F32R, name=f"V{i}", tag=f"V{i}") for i in range(ST)]

        for h in range(HL):
            nc.sync.dma_start(QTa[h][64:65, :], onesd[:].bitcast(F32R))
            nc.sync.dma_start(KTa[h][64:65, :], mbias[:].bitcast(F32R))

        # ---- Phase A: input projections ----
        with ExitStack() as phA:
            wpool = phA.enter_context(tc.tile_pool(name="wpool", bufs=2))
            xpool = phA.enter_context(tc.tile_pool(name="xpool", bufs=KC + 3))
            ppqk = phA.enter_context(tc.tile_pool(name="ppqk", bufs=3, space="PSUM"))
            ppv = phA.enter_context(tc.tile_pool(name="ppv", bufs=3, space="PSUM"))

            for which, xdram, wdram in (() if no_a else (("q", qT, wqT), ("k", kT, wkT), ("v", vT, wvT))):
                w_sb = wpool.tile([128, KC, DL], F32R, name="w", tag="w")
                nc.sync.dma_start(w_sb[:], wdram[:].rearrange("(c p) d -> p c d", p=128).bitcast(F32R))
                xts = []
                for c in range(KC):
                    xt = xpool.tile([128, S], F32R, name="xT", tag="xT")
                    nc.sync.dma_start(xt[:], xdram[c * 128:(c + 1) * 128, :].bitcast(F32R))
                    xts.append(xt)
                if which in ("q", "k"):
                    dsts = QTa if which == "q" else KTa
                    bias = bq_sb if which == "q" else bk_sb
                    scale = 0.125 if which == "q" else 1.0
                    for h in range(HL):
                        for sc in range(NW):
                            w512 = min(512, S)
                            ps = ppqk.tile([DK, 512], F32, name="pqk", tag="pqk")
                            for c in range(KC):
                                nc.tensor.matmul(
                                    ps[:, 0:w512],
                                    w_sb[:, c, h * DK:(h + 1) * DK],
                                    xts[c][:, sc * w512:(sc + 1) * w512],
                                    start=(c == 0), stop=(c == KC - 1),
                                )
                            nc.scalar.activation(
                                dsts[h][0:DK, sc * w512:(sc + 1) * w512],
                                ps[:, 0:w512],
                                AF.Identity, bias=bias[:, h:h + 1], scale=scale,
                            )
                else:
                    for st in range(ST):
                        ps = ppv.tile([128, DL], F32, name="pv", tag="pv")
                        for c in range(KC):
                            nc.tensor.matmul(
                                ps[:], xts[c][:, st * 128:(st + 1) * 128],
                                w_sb[:, c, :],
                                start=(c == 0), stop=(c == KC - 1),
                            )
                        nc.vector.tensor_add(V_sb[st][:], ps[:], bvb_sb[:])

        pers2 = top.enter_context(tc.tile_pool(name="pers2", bufs=1))
        OT_sb = [pers2.tile([64, S], F32R, name=f"OT{i}", tag=f"OT{i}") for i in range(HL)]

        # ---- Phase B: attention ----
        with ExitStack() as phB:
            psc = phB.enter_context(tc.tile_pool(name="psc", bufs=3, space="PSUM"))
            ppt = phB.enter_context(tc.tile_pool(name="ppt", bufs=2, space="PSUM"))
            ppo = phB.enter_context(tc.tile_pool(name="ppo", bufs=1, space="PSUM"))
            pP = phB.enter_context(tc.tile_pool(name="pP", bufs=2))
            pPn = phB.enter_context(tc.tile_pool(name="pPn", bufs=2))
            pPT = phB.enter_context(tc.tile_pool(name="pPT", bufs=1))
            pstat = phB.enter_context(tc.tile_pool(name="pstat", bufs=4))
            ppsr = phB.enter_context(tc.tile_pool(name="ppsr", bufs=1, space="PSUM"))
            ppbc = phB.enter_context(tc.tile_pool(name="ppbc", bufs=1, space="PSUM"))
            prr = phB.enter_context(tc.tile_pool(name="prr", bufs=2))

            for h in range(0 if no_b else HL):
                for g in range(ST // NG):
                    PT = pPT.tile([128, KT2, NG, 128], F32R, name="PT", tag="PT")
                    psr = ppsr.tile([1, NG * 128], F32, name="psr", tag="psr")
                    for qq in range(NG):
                        qt = g * NG + qq
                        P = pP.tile([128, S], F32R, name="P", tag="P")
                        rs = pstat.tile([128, NPS + 2], F32, name="rs", tag="rs")
                        for cc in range(NPS):
                            ps = psc.tile([128, PW], F32, name="sc", tag="sc")
                            for j in range(PW // 512):
                                nc.tensor.matmul(
                                    ps[:, j * 512:(j + 1) * 512],
                                    QTa[h][:, qt * 128:(qt + 1) * 128],
                                    KTa[h][:, cc * PW + j * 512: cc * PW + (j + 1) * 512],
                                    start=True, stop=True,
                                )
                            if no_sm:
                                nc.scalar.activation(P[:, cc * PW:(cc + 1) * PW], ps[:], AF.Copy)
                            else:
                                nc.scalar.activation(
                                    P[:, cc * PW:(cc + 1) * PW], ps[:], AF.Exp,
                                    accum_out=rs[:, cc:cc + 1],
                                )
                        Pn = pPn.tile([128, S], F32, name="Pn", tag="Pn")
                        if no_sm:
                            nc.vector.tensor_copy(Pn[:], P[:])
                        else:
                            nc.vector.reduce_sum(rs[:, NPS:NPS + 1], rs[:, 0:NPS], axis=AX.X)
                            nc.vector.reciprocal(rs[:, NPS + 1:NPS + 2], rs[:, NPS:NPS + 1])
                            nc.vector.tensor_scalar_mul(Pn[:], P.bitcast(F32)[:], rs[:, NPS + 1:NPS + 2])
                        if not no_attn_dma:
                            nc.gpsimd.dma_start(attn_o[h, qt * 128:(qt + 1) * 128, :], Pn[:])
                        if not no_pv:
                            nc.tensor.transpose(
                                psr[0:1, qq * 128:(qq + 1) * 128],
                                rs[:, NPS + 1:NPS + 2], ident[:],
                            )
                        for j in range(0 if no_pv else (KT2 // 4)):
                            pt = ppt.tile([128, 512], F32R, name="pt", tag="pt")
                            for i in range(4):
                                nc.tensor.transpose(
                                    pt[:, i * 128:(i + 1) * 128],
                                    P[:, (4 * j + i) * 128:(4 * j + i + 1) * 128],
                                    identr[:],
                                )
                            src = pt[:].rearrange("p (a b) -> p a b", a=4)
                            if j % 2 == 0:
                                nc.vector.tensor_copy(PT[:, 4 * j:4 * j + 4, qq, :], src)
                            else:
                                nc.scalar.activation(PT[:, 4 * j:4 * j + 4, qq, :], src, AF.Copy)
                    if no_pv:
                        continue
                    rrow = prr.tile([1, NG * 128], F32R, name="rrow", tag="rrow")
                    nc.vector.tensor_copy(rrow[:], psr[:])
                    pbc = ppbc.tile([64, NG * 128], F32, name="pbc", tag="pbc")
                    nc.tensor.matmul(pbc[:], ones_sb[:], rrow[:], start=True, stop=True)
                    rbc = prr.tile([64, NG * 128], F32, name="rbc", tag="rbc")
                    nc.scalar.activation(rbc[:], pbc[:], AF.Copy)
                    po = ppo.tile([64, NG * 128], F32, name="po", tag="po")
                    for c2 in range(KT2):
                        nc.tensor.matmul(
                            po[:],
                            V_sb[c2][:, h * DK:(h + 1) * DK],
                            PT[:, c2, :, :],
                            start=(c2 == 0), stop=(c2 == KT2 - 1),
                        )
                    nc.vector.tensor_mul(
                        OT_sb[h][:, g * NG * 128:(g + 1) * NG * 128],
                        po[:],
                        rbc[:],
                    )

        # ---- Phase C: partial output projection ----
        with ExitStack() as phC:
            pout = phC.enter_context(tc.tile_pool(name="pout", bufs=2, space="PSUM"))
            pos = phC.enter_context(tc.tile_pool(name="pos", bufs=3))
            ST2 = 0 if (no_proj or no_pv) else (S // 128)
            for st in range(ST2):
                ot = pos.tile([128, H], F32, name="ot", tag="ot")
                for oc in range(H // 512):
                    ps = pout.tile([128, 512], F32, name="po2", tag="po2")
                    for hp in range(HL):
                        nc.tensor.matmul(
                            ps[:],
                            OT_sb[hp][:, st * 128:(st + 1) * 128],
                            wo_sb[hp][:, oc * 512:(oc + 1) * 512],
                            start=(hp == 0), stop=(hp == HL - 1),
                        )
                    nc.scalar.activation(ot[:, oc * 512:(oc + 1) * 512], ps[:], AF.Copy)
                nc.gpsimd.dma_start(out_o[st * 128:(st + 1) * 128, :], ot[:])

    nc.compile()
    return nc


_NC_CACHE = {}


def _get_nc():
    if "full" not in _NC_CACHE:
        _NC_CACHE["full"] = build_nc()
    return _NC_CACHE["full"]


def _prep_core_inputs(c, q, k, v, mask, WqT, WkT, WvT, WoT, Wq_b, Wk_b, Wv_b):
    b, hg = c // 4, c % 4
    sl = slice(hg * DL, (hg + 1) * DL)
    mb = np.where(mask[b] == 0, np.float32(NEG), np.float32(0.0))
    return {
        "qT": np.ascontiguousarray(q[b].T),
        "kT": np.ascontiguousarray(k[b].T),
        "vT": np.ascontiguousarray(v[b].T),
        "wqT": np.ascontiguousarray(WqT[:, sl]),
        "wkT": np.ascontiguousarray(WkT[:, sl]),
        "wvT": np.ascontiguousarray(WvT[:, sl]),
        "woT": np.ascontiguousarray(WoT[sl, :]),
        "bqs": np.ascontiguousarray((Wq_b[sl] / 8.0).reshape(HL, DK).T),
        "bks": np.ascontiguousarray(Wk_b[sl].reshape(HL, DK).T),
        "bvb": np.ascontiguousarray(np.broadcast_to(Wv_b[sl], (128, DL))),
        "mbias": np.ascontiguousarray(mb.reshape(1, S)),
        "onesd": np.ones((1, S), np.float32),
        "eyed": np.eye(128, dtype=np.float32),
    }


def kernel(q, k, v, mask, Wq_w, Wq_b, Wk_w, Wk_b, Wv_w, Wv_b, Wo_w, Wo_b,
           _trace=False):
    q = np.asarray(q, np.float32)
    k = np.asarray(k, np.float32)
    v = np.asarray(v, np.float32)
    mask = np.asarray(mask)
    WqT = np.ascontiguousarray(np.asarray(Wq_w, np.float32).T)
    WkT = np.ascontiguousarray(np.asarray(Wk_w, np.float32).T)
    WvT = np.ascontiguousarray(np.asarray(Wv_w, np.float32).T)
    WoT = np.ascontiguousarray(np.asarray(Wo_w, np.float32).T)
    Wq_b = np.asarray(Wq_b, np.float32)
    Wk_b = np.asarray(Wk_b, np.float32)
    Wv_b = np.asarray(Wv_b, np.float32)
    Wo_b = np.asarray(Wo_b, np.float32)

    nc = _get_nc()
    in_maps = [
        _prep_core_inputs(c, q, k, v, mask, WqT, WkT, WvT, WoT, Wq_b, Wk_b, Wv_b)
        for c in range(8)
    ]
    res = run_bass_kernel_spmd(nc, in_maps, core_ids=list(range(8)), trace=_trace)

    out = np.zeros((B, S, H), np.float32)
    attn = np.empty((B, NH, S, S), np.float32)
    for c in range(8):
        b, hg = c // 4, c % 4
        out[b] += res.results[c]["out_o"]
        attn[b, hg * HL:(hg + 1) * HL] = res.results[c]["attn_o"]
    out += Wo_b[None, None, :]
    if _trace:
        kernel._last_results = res
    return out, attn
